# revision 1
# baseline (speedup 1.0000x reference)
"""GNN GRU message-passing kernel for 8 Trainium2 NeuronCores.

Design:
  - Nodes padded to 12800/core (8 cores, 102400 padded global rows).
  - Per layer: msg = h @ W computed per-shard (PE), AllGather -> msg_full
    (DRAM), per-edge gather via indirect DMA (128 edges/op, one offset per
    SBUF partition), segment-sum into m^T via PE one-hot matmuls, GRU cell
    computed feature-major on PE/ACT/DVE.
  - Edges partitioned by dst shard; grouped by 128-wide dst blocks; the
    (block -> op range) schedule is uniform across cores (SPMD), per-core
    differences live in the index/one-hot input data only.
"""
import numpy as np

import concourse.bass as bass
import concourse.bacc as bacc
import concourse.tile as tile
from concourse.bass_utils import run_bass_kernel_spmd

mybir = bass.mybir

NCORES = 8
N_NODES = 100000
SHARD = 12500          # real nodes per core
PAD_SH = 12800         # padded nodes per core (100 blocks of 128)
NBLK = PAD_SH // 128   # 100 dst blocks per core
NTOT = PAD_SH * NCORES
C = 64
N_LAYERS = 10
GRU_CHUNK = 512
NGRU = PAD_SH // GRU_CHUNK  # 25
PADV = 999.0           # one-hot miss sentinel for pad edges


def _preprocess(edge_index):
    """Returns (gsrc [8,128,NOPS] i32, dstrel [8,128,NOPS] f32, opb [NBLK])."""
    src = np.asarray(edge_index[0], dtype=np.int64)
    dst = np.asarray(edge_index[1], dtype=np.int64)
    ps = (src // SHARD) * PAD_SH + (src % SHARD)   # padded global src id
    core = dst // SHARD
    dl = dst % SHARD                               # local dst id
    blk = dl // 128

    # edge counts per (core, block)
    cnt = np.zeros((NCORES, NBLK), np.int64)
    np.add.at(cnt, (core, blk), 1)
    opb = np.maximum(1, -(-cnt.max(axis=0) // 128))     # ops per block
    nops = int(opb.sum())
    op_base = np.concatenate([[0], np.cumsum(opb)])[:-1]  # first op of block

    gsrc = np.zeros((NCORES, 128, nops), np.int32)
    dstrel = np.full((NCORES, 128, nops), PADV, np.float32)
    for c in range(NCORES):
        m = core == c
        o = np.argsort(blk[m], kind="stable")
        cps, cdl, cblk = ps[m][o], dl[m][o], blk[m][o]
        # position of each edge within its block
        pos = np.arange(cps.size) - np.repeat(
            np.concatenate([[0], np.cumsum(cnt[c])])[:-1], cnt[c])
        tok = (op_base[cblk] + pos // 128) * 128 + pos % 128
        g = gsrc[c].T.reshape(-1)      # token t -> [t%128, t//128] => use [t//... ]
        # token index t maps to op k=t//128, partition p=t%128 -> gsrc[c][p, k]
        gsrc[c][tok % 128, tok // 128] = cps.astype(np.int32)
        dstrel[c][tok % 128, tok // 128] = (cdl - cblk * 128).astype(np.float32)
    return gsrc, dstrel, opb


def _build(nops, opb, nlayers=1):
    nc = bacc.Bacc("TRN2", target_bir_lowering=False, debug=False,
                   num_devices=NCORES)
    f32 = mybir.dt.float32
    din = lambda n, s, d=f32: nc.dram_tensor(n, s, d, kind="ExternalInput")
    xT_in = din("xT", [C, PAD_SH])
    w_in = din("wstack", [C, 2 * C])
    rsel_in = din("rsel", [128, 1])
    wihT_in = din("wihT", [C, 3 * C])
    whhT_in = din("whhT", [C, 3 * C])
    brz_in = din("brz", [C, 2])
    bihn_in = din("bihn", [C, 1])
    bhhn_in = din("bhhn", [C, 1])
    iota_in = din("iota", [128, 128])
    ident_in = din("ident", [C, C])
    gsrc_in = din("gsrc", [128, nops], mybir.dt.int32)
    drel_in = din("drel", [128, nops])
    out = nc.dram_tensor("hout", [PAD_SH, C], f32, kind="ExternalOutput")

    op_base = np.concatenate([[0], np.cumsum(opb)])[:-1]

    with tile.TileContext(nc) as tc:
        with tc.tile_pool(name="dram", bufs=1, space="DRAM") as dram, \
             tc.tile_pool(name="persist", bufs=1) as pp, \
             tc.tile_pool(name="work", bufs=16) as wp, \
             tc.tile_pool(name="stage", bufs=3) as sp, \
             tc.tile_pool(name="psum", bufs=1, space="PSUM") as psp:
            msg_shard = dram.tile([PAD_SH, C], f32)
            msg_full = dram.tile([NTOT, C], f32, addr_space="Shared")

            hT = pp.tile([C, PAD_SH], f32)
            mT = pp.tile([C, PAD_SH], f32)
            wstack = pp.tile([C, 2 * C], f32)
            rsel = pp.tile([128, 1], f32)
            wihT = pp.tile([C, 3 * C], f32)
            whhT = pp.tile([C, 3 * C], f32)
            brz = pp.tile([C, 2], f32)
            bihn = pp.tile([C, 1], f32)
            bhhn = pp.tile([C, 1], f32)
            iota = pp.tile([128, 128], f32)
            ident = pp.tile([C, C], f32)
            gsrc = pp.tile([128, nops], mybir.dt.int32)
            drel = pp.tile([128, nops], f32)

            for t, i in [(hT, xT_in), (wstack, w_in), (wihT, wihT_in),
                         (whhT, whhT_in), (brz, brz_in), (bihn, bihn_in),
                         (bhhn, bhhn_in), (iota, iota_in), (ident, ident_in),
                         (gsrc, gsrc_in), (drel, drel_in), (rsel, rsel_in)]:
                nc.sync.dma_start(t[:], i.ap())

            AF = mybir.ActivationFunctionType
            OP = mybir.AluOpType

            def emit_msg_and_ag(l):
                # msg chunks node-major -> msg_shard -> AllGather
                for grp in range(NBLK // 4):         # 4 sub-blocks staged
                    stg = sp.tile([128, 4, C], f32, tag="msgstage")
                    for k in range(4):
                        sub = grp * 4 + k
                        pm = psp.tile([128, C], f32, tag="pmsg", bufs=2)
                        nc.tensor.matmul(
                            pm[:], hT[:, sub * 128:(sub + 1) * 128],
                            wstack[:, l * C:(l + 1) * C], start=True, stop=True)
                        nc.scalar.activation(stg[:, k], pm[:], AF.Copy)
                    nc.sync.dma_start(
                        msg_shard[grp * 512:(grp + 1) * 512].rearrange(
                            "(a p) c -> p a c", p=128), stg[:])
                nc.gpsimd.collective_compute(
                    "AllGather", mybir.AluOpType.bypass,
                    replica_groups=[list(range(NCORES))],
                    ins=[msg_shard[:]], outs=[msg_full[:]])

            def emit_edges():
                for g in range(NBLK):
                    pseg = psp.tile([C, 128], f32, tag="pseg", bufs=2)
                    nc.vector.memset(pseg[:], 0.0)
                    with tc.For_i(0, 2):
                        for j in range(int(opb[g])):
                            k = int(op_base[g]) + j
                            gt = wp.tile([128, C], f32, tag="g")
                            nc.gpsimd.indirect_dma_start(
                                gt[:], None, msg_full[:],
                                bass.IndirectOffsetOnAxis(ap=gsrc[:, k:k + 1], axis=0))
                            oh = wp.tile([128, 128], f32, tag="oh")
                            nc.vector.tensor_scalar(
                                oh[:], iota[:], drel[:, k:k + 1], 0.5,
                                OP.is_equal, OP.mult)
                            nc.tensor.matmul(pseg[:], gt[:], oh[:],
                                             start=False, stop=False,
                                             skip_group_check=True)
                    nc.scalar.activation(mT[:, g * 128:(g + 1) * 128], pseg[:],
                                         AF.Copy)

            def emit_gru():
                for cch in range(NGRU):
                    s, e = cch * GRU_CHUNK, (cch + 1) * GRU_CHUNK
                    mch, hch = mT[:, s:e], hT[:, s:e]
                    pr = psp.tile([C, GRU_CHUNK], f32, tag="pr")
                    pz = psp.tile([C, GRU_CHUNK], f32, tag="pz")
                    pni = psp.tile([C, GRU_CHUNK], f32, tag="pni")
                    pnh = psp.tile([C, GRU_CHUNK], f32, tag="pnh")
                    nc.tensor.matmul(pr[:], wihT[:, 0:C], mch, start=True, stop=False)
                    nc.tensor.matmul(pr[:], whhT[:, 0:C], hch, start=False, stop=True)
                    nc.tensor.matmul(pz[:], wihT[:, C:2 * C], mch, start=True, stop=False)
                    nc.tensor.matmul(pz[:], whhT[:, C:2 * C], hch, start=False, stop=True)
                    nc.tensor.matmul(pni[:], wihT[:, 2 * C:3 * C], mch, start=True, stop=True)
                    nc.tensor.matmul(pnh[:], whhT[:, 2 * C:3 * C], hch, start=True, stop=True)
                    r = sp.tile([C, GRU_CHUNK], f32, tag="r")
                    z = sp.tile([C, GRU_CHUNK], f32, tag="z")
                    hnb = sp.tile([C, GRU_CHUNK], f32, tag="hnb")
                    t1 = sp.tile([C, GRU_CHUNK], f32, tag="t1")
                    n = sp.tile([C, GRU_CHUNK], f32, tag="n")
                    d = sp.tile([C, GRU_CHUNK], f32, tag="d")
                    nc.scalar.activation(r[:], pr[:], AF.Sigmoid, bias=brz[:, 0:1])
                    nc.scalar.activation(z[:], pz[:], AF.Sigmoid, bias=brz[:, 1:2])
                    nc.vector.tensor_scalar(hnb[:], pnh[:], bhhn[:, 0:1], None, OP.add)
                    nc.vector.tensor_tensor(t1[:], r[:], hnb[:], OP.mult)
                    nc.vector.tensor_tensor(t1[:], t1[:], pni[:], OP.add)
                    nc.scalar.activation(n[:], t1[:], AF.Tanh, bias=bihn[:])
                    nc.vector.tensor_tensor(d[:], hch, n[:], OP.subtract)
                    nc.vector.tensor_tensor(d[:], z[:], d[:], OP.mult)
                    nc.vector.tensor_tensor(hch, n[:], d[:], OP.add)

            for l in range(nlayers):
                emit_msg_and_ag(l)
                emit_edges()
                emit_gru()

            # final relu + transpose to node-major + store
            for grp in range(NBLK // 4):
                stg = sp.tile([128, 4, C], f32, tag="outstage")
                for k in range(4):
                    sub = grp * 4 + k
                    pt = psp.tile([128, C], f32, tag="pmsg", bufs=2)
                    nc.tensor.matmul(pt[:], hT[:, sub * 128:(sub + 1) * 128],
                                     ident[:], start=True, stop=True)
                    nc.scalar.activation(stg[:, k], pt[:], AF.Copy)
                    tmp = sp.tile([128, C], f32, tag="otmp")
                    nc.vector.tensor_scalar_mul(tmp[:], stg[:, k], rsel[:, 0:1])
                    nc.vector.tensor_tensor(stg[:, k], stg[:, k], tmp[:], OP.max)
                nc.sync.dma_start(
                    out.ap()[grp * 512:(grp + 1) * 512].rearrange(
                        "(a p) c -> p a c", p=128), stg[:])
    nc.compile()
    return nc


_CACHE = {}


def kernel(x, edge_index, weight, w_ih, w_hh, b_ih, b_hh):
    x = np.asarray(x, np.float32)
    weight = np.asarray(weight, np.float32)
    w_ih = np.asarray(w_ih, np.float32)
    w_hh = np.asarray(w_hh, np.float32)
    b_ih = np.asarray(b_ih, np.float32)
    b_hh = np.asarray(b_hh, np.float32)

    gsrc, dstrel, opb = _preprocess(edge_index)
    nops = int(opb.sum())

    key = ("k2", nops, tuple(opb.tolist()))
    if key not in _CACHE:
        _CACHE[key] = _build(nops, opb)
    nc = _CACHE[key]

    xpad = np.zeros((NCORES, PAD_SH, C), np.float32)
    xr = x.reshape(NCORES, SHARD, C)
    xpad[:, :SHARD] = xr

    wihT = w_ih.T.copy()                                   # [64, 192]
    whhT = w_hh.T.copy()
    brz = np.stack([(b_ih + b_hh)[0:C], (b_ih + b_hh)[C:2 * C]], 1)  # [64,2]
    bihn = b_ih[2 * C:3 * C].reshape(C, 1).copy()
    bhhn = b_hh[2 * C:3 * C].reshape(C, 1).copy()
    iota = np.tile(np.arange(128, dtype=np.float32), (128, 1))
    ident = np.eye(C, dtype=np.float32)

    h = xpad  # [8, PAD_SH, C]
    for step in range(N_LAYERS):
        wstack = np.concatenate([weight[step], weight[step]], axis=1)
        last = step == N_LAYERS - 1
        rsel = np.full((128, 1), 0.0 if last else 1.0, np.float32)
        in_maps = []
        for c in range(NCORES):
            in_maps.append({
                "xT": h[c].T.copy(), "wstack": wstack, "wihT": wihT,
                "whhT": whhT, "brz": brz, "bihn": bihn, "bhhn": bhhn,
                "iota": iota, "ident": ident, "rsel": rsel,
                "gsrc": gsrc[c], "drel": dstrel[c],
            })
        res = run_bass_kernel_spmd(nc, in_maps, core_ids=list(range(NCORES)),
                                   trace=False)
        hn = np.zeros((NCORES, PAD_SH, C), np.float32)
        for c in range(NCORES):
            hn[c, :SHARD] = res.results[c]["hout"][:SHARD]
        h = hn
    return h[:, :SHARD].reshape(N_NODES, C)



# revision 13
# speedup vs baseline: 849.6078x; 849.6078x over previous
"""GNN GRU message-passing kernel for 8 Trainium2 NeuronCores.

Single-launch pipelined design (v3):
  - Nodes padded to 12800/core (8 cores, 102400 padded global rows).
  - All 10 layers unrolled in one device program, one kernel launch.
  - Per layer: msg = h @ W_l per-shard (PE, fp32 in / bf16 out), exchanged
    via TWO half-shard AllGathers (bf16) fired as soon as their message
    rows are ready, so collective latency hides behind the previous
    layer's gather phase. Edges are classed by src half; a dst block's
    ops list class-0 ops then class-1 ops, all accumulating one PSUM.
  - Per-edge gather via indirect DMA (128 edges/op, one offset per SBUF
    partition, bf16 rows); segment-sum into mT via PE one-hot matmuls
    (bf16 x bf16 -> fp32 PSUM); one-hot for a block's ops generated in a
    single DVE is_equal with stride-0 broadcast APs.
  - Emission is software-pipelined: after each 512-node span's 4 dst
    blocks, its GRU chunk runs; completed 640-node spans emit the next
    layer's msg matmuls; AllGather halves fire mid-phase.
"""
import numpy as np

import concourse.bass as bass
import concourse.bacc as bacc
import concourse.tile as tile
from concourse.bass_utils import run_bass_kernel_spmd

mybir = bass.mybir

NCORES = 8
N_NODES = 100000
SHARD = 12500          # real nodes per core
PAD_SH = 12800         # padded nodes per core (100 blocks of 128)
NBLK = PAD_SH // 128   # 100 dst blocks per core
HALF = 6400            # src-half rows per core (class chunking)
HTAB = HALF * NCORES   # 51200 rows per half table
C = 64
N_LAYERS = 10
GRU_CHUNK = 512
NGRU = PAD_SH // GRU_CHUNK   # 25
MSG_GRP = 640
NMSG = PAD_SH // MSG_GRP     # 20 (10 per half)
PADV = 999.0           # one-hot miss sentinel for pad edges


def _preprocess(edge_index):
    """Edge schedule with src-half classes.

    Returns (gsrc [8,128,NOPS] i32 rows into the half tables,
             dstrel [8,128,NOPS] f32, opb2 [NBLK,2] ops per block/class).
    """
    src = np.asarray(edge_index[0], dtype=np.int64)
    dst = np.asarray(edge_index[1], dtype=np.int64)
    sloc = src % SHARD
    half = sloc // HALF                      # 0 or 1 (sloc < 12500 < 2*6400)
    trow = (src // SHARD) * HALF + sloc % HALF   # row in its half table
    core = dst // SHARD
    dl = dst % SHARD
    blk = dl // 128

    cnt = np.zeros((NCORES, NBLK, 2), np.int64)
    np.add.at(cnt, (core, blk, half), 1)
    opb2 = np.maximum(1, -(-cnt.max(axis=0) // 128))   # [NBLK, 2]
    opb = opb2.sum(axis=1)
    nops = int(opb.sum())
    # op column base for (block, class): class-0 ops then class-1 ops
    blk_base = np.concatenate([[0], np.cumsum(opb)])[:-1]
    cls_base = np.stack([blk_base, blk_base + opb2[:, 0]], 1)  # [NBLK,2]

    gsrc = np.zeros((NCORES, 128, nops), np.int32)
    dstrel = np.full((NCORES, 128, nops), PADV, np.float32)
    key = blk * 2 + half
    ncnt_flat = cnt.reshape(NCORES, NBLK * 2)
    for c in range(NCORES):
        m = core == c
        o = np.argsort(key[m], kind="stable")
        ctr, cdl, ck = trow[m][o], dl[m][o], key[m][o]
        pos = np.arange(ctr.size) - np.repeat(
            np.concatenate([[0], np.cumsum(ncnt_flat[c])])[:-1], ncnt_flat[c])
        base = cls_base.reshape(-1)[ck]
        tok = (base + pos // 128) * 128 + pos % 128
        gsrc[c][tok % 128, tok // 128] = ctr.astype(np.int32)
        dstrel[c][tok % 128, tok // 128] = (cdl - (ck // 2) * 128).astype(
            np.float32)
    return gsrc, dstrel, opb2


def _build(nops, opb2, nlayers=N_LAYERS):
    nc = bacc.Bacc("TRN2", target_bir_lowering=False, debug=False,
                   num_devices=NCORES)
    f32 = mybir.dt.float32
    bf16 = mybir.dt.bfloat16
    din = lambda n, s, d=f32: nc.dram_tensor(n, s, d, kind="ExternalInput")
    xT_in = din("xT", [C, PAD_SH])
    w_in = din("wstack", [C, nlayers * C])
    wihT_in = din("wihT", [C, 3 * C])
    whhT_in = din("whhT", [C, 3 * C])
    brz_in = din("brz", [C, 2])
    bihn_in = din("bihn", [C, 1])
    bhhn_in = din("bhhn", [C, 1])
    iota_in = din("iota", [128, 128])
    ident_in = din("ident", [C, C])
    gsrc_in = din("gsrc", [128, nops], mybir.dt.int32)
    drel_in = din("drel", [128, nops])
    out = nc.dram_tensor("hout", [PAD_SH, C], f32, kind="ExternalOutput")

    opb = opb2.sum(axis=1)
    blk_base = np.concatenate([[0], np.cumsum(opb)])[:-1]

    with tile.TileContext(nc) as tc:
        with tc.tile_pool(name="dram", bufs=1, space="DRAM") as dram, \
             tc.tile_pool(name="persist", bufs=1) as pp, \
             tc.tile_pool(name="work", bufs=16) as wp, \
             tc.tile_pool(name="ohpool", bufs=3) as ohp, \
             tc.tile_pool(name="stage", bufs=3) as sp, \
             tc.tile_pool(name="psum", bufs=1, space="PSUM") as psp:
            mshs, mfus = [], []
            for l in range(nlayers):
                row = []
                for h in range(2):
                    msh = dram.tile([HALF, C], bf16, tag=f"msh{l}_{h}")
                    mfu = dram.tile([HTAB, C], bf16, addr_space="Shared",
                                    tag=f"mfu{l}_{h}")
                    row.append((msh, mfu))
                mshs.append([r[0] for r in row])
                mfus.append([r[1] for r in row])

            hT = pp.tile([C, PAD_SH], f32)
            mT = pp.tile([C, PAD_SH], f32)
            wstack = pp.tile([C, nlayers * C], f32)
            wihT = pp.tile([C, 3 * C], f32)
            whhT = pp.tile([C, 3 * C], f32)
            brz = pp.tile([C, 2], f32)
            bihn = pp.tile([C, 1], f32)
            bhhn = pp.tile([C, 1], f32)
            iota = pp.tile([128, 128], f32)
            ident = pp.tile([C, C], f32)
            gsrc = pp.tile([128, nops], mybir.dt.int32)
            drel = pp.tile([128, nops], f32)

            for t, i in [(hT, xT_in), (wstack, w_in), (wihT, wihT_in),
                         (whhT, whhT_in), (brz, brz_in), (bihn, bihn_in),
                         (bhhn, bhhn_in), (iota, iota_in), (ident, ident_in),
                         (gsrc, gsrc_in), (drel, drel_in)]:
                nc.sync.dma_start(t[:], i.ap())

            AF = mybir.ActivationFunctionType
            OP = mybir.AluOpType

            def emit_msg_group(l, m):
                # 640 msg rows (5 sub-blocks of 128) for layer l
                stg = sp.tile([128, 5, C], bf16, tag="msgstage")
                for k in range(5):
                    sub = m * 5 + k
                    pm = psp.tile([128, C], f32, tag="pmsg", bufs=2)
                    nc.tensor.matmul(
                        pm[:], hT[:, sub * 128:(sub + 1) * 128],
                        wstack[:, l * C:(l + 1) * C], start=True, stop=True)
                    nc.scalar.activation(stg[:, k], pm[:], AF.Copy)
                h = m // (NMSG // 2)
                r0 = (m % (NMSG // 2)) * MSG_GRP
                nc.sync.dma_start(
                    mshs[l][h][r0:r0 + MSG_GRP].rearrange(
                        "(a p) c -> p a c", p=128), stg[:])

            def emit_ag(l, h):
                nc.gpsimd.collective_compute(
                    "AllGather", mybir.AluOpType.bypass,
                    replica_groups=[list(range(NCORES))],
                    ins=[mshs[l][h][:]], outs=[mfus[l][h][:]])

            def emit_block(l, g, cls):
                # cls 0: class-0 ops -> mT copy; cls 1: class-1 ops -> mT add
                n0, n1 = int(opb2[g, 0]), int(opb2[g, 1])
                nop = n1 if cls else n0
                k0 = int(blk_base[g]) + (n0 if cls else 0)
                oh = ohp.tile([128, nop * 128], bf16, tag="oh")
                nc.vector.tensor_tensor(
                    oh[:],
                    iota[:].rearrange("p (j s) -> p j s", j=1).to_broadcast(
                        [128, nop, 128]),
                    drel[:, k0:k0 + nop].rearrange("p (j s) -> p j s", s=1)
                    .to_broadcast([128, nop, 128]),
                    OP.is_equal)
                pseg = psp.tile([C, 128], f32, tag="pseg", bufs=2)
                for j in range(nop):
                    k = k0 + j
                    gt = wp.tile([128, C], bf16, tag="g")
                    nc.gpsimd.indirect_dma_start(
                        gt[:], None, mfus[l][cls][:],
                        bass.IndirectOffsetOnAxis(ap=gsrc[:, k:k + 1], axis=0))
                    nc.tensor.matmul(pseg[:], gt[:],
                                     oh[:, j * 128:(j + 1) * 128],
                                     start=(j == 0), stop=(j == nop - 1),
                                     skip_group_check=True)
                dst = mT[:, g * 128:(g + 1) * 128]
                if cls == 0:
                    nc.scalar.activation(dst, pseg[:], AF.Copy)
                else:
                    nc.vector.tensor_tensor(dst, dst, pseg[:], OP.add)

            def emit_gru_chunk(cch):
                s, e = cch * GRU_CHUNK, (cch + 1) * GRU_CHUNK
                mch, hch = mT[:, s:e], hT[:, s:e]
                pr = psp.tile([C, GRU_CHUNK], f32, tag="pr")
                pz = psp.tile([C, GRU_CHUNK], f32, tag="pz")
                pni = psp.tile([C, GRU_CHUNK], f32, tag="pni")
                pnh = psp.tile([C, GRU_CHUNK], f32, tag="pnh")
                nc.tensor.matmul(pr[:], wihT[:, 0:C], mch, start=True, stop=False)
                nc.tensor.matmul(pr[:], whhT[:, 0:C], hch, start=False, stop=True)
                nc.tensor.matmul(pz[:], wihT[:, C:2 * C], mch, start=True, stop=False)
                nc.tensor.matmul(pz[:], whhT[:, C:2 * C], hch, start=False, stop=True)
                nc.tensor.matmul(pni[:], wihT[:, 2 * C:3 * C], mch, start=True, stop=True)
                nc.tensor.matmul(pnh[:], whhT[:, 2 * C:3 * C], hch, start=True, stop=True)
                r = sp.tile([C, GRU_CHUNK], f32, tag="r")
                z = sp.tile([C, GRU_CHUNK], f32, tag="z")
                hnb = sp.tile([C, GRU_CHUNK], f32, tag="hnb")
                t1 = sp.tile([C, GRU_CHUNK], f32, tag="t1")
                n = sp.tile([C, GRU_CHUNK], f32, tag="n")
                d = sp.tile([C, GRU_CHUNK], f32, tag="d")
                nc.scalar.activation(r[:], pr[:], AF.Sigmoid, bias=brz[:, 0:1])
                nc.scalar.activation(z[:], pz[:], AF.Sigmoid, bias=brz[:, 1:2])
                nc.vector.tensor_scalar(hnb[:], pnh[:], bhhn[:, 0:1], None, OP.add)
                nc.vector.tensor_tensor(t1[:], r[:], hnb[:], OP.mult)
                nc.vector.tensor_tensor(t1[:], t1[:], pni[:], OP.add)
                nc.scalar.activation(n[:], t1[:], AF.Tanh, bias=bihn[:])
                nc.vector.tensor_tensor(d[:], hch, n[:], OP.subtract)
                nc.vector.tensor_tensor(d[:], z[:], d[:], OP.mult)
                nc.vector.tensor_tensor(hch, n[:], d[:], OP.add)

            # prologue: layer-0 messages from the initial h, both AG halves
            for m in range(NMSG):
                emit_msg_group(0, m)
                if m == NMSG // 2 - 1:
                    emit_ag(0, 0)
            emit_ag(0, 1)

            for l in range(nlayers):
                # pass 0: class-0 gathers of all blocks (hides AG half-1)
                for g in range(NBLK):
                    emit_block(l, g, 0)
                # pass 1: class-1 gathers + GRU + next-layer msgs interleaved
                next_m = 0
                for cch in range(NGRU):
                    for g in range(cch * 4, cch * 4 + 4):
                        emit_block(l, g, 1)
                    emit_gru_chunk(cch)
                    if l + 1 < nlayers:
                        while (next_m < NMSG and
                               (next_m + 1) * MSG_GRP <= (cch + 1) * GRU_CHUNK):
                            emit_msg_group(l + 1, next_m)
                            if next_m == NMSG // 2 - 1:
                                emit_ag(l + 1, 0)
                            next_m += 1
                if l + 1 < nlayers:
                    emit_ag(l + 1, 1)

            # final relu + transpose to node-major + store
            for grp in range(NBLK // 4):
                stg = sp.tile([128, 4, C], f32, tag="outstage")
                for k in range(4):
                    sub = grp * 4 + k
                    pt = psp.tile([128, C], f32, tag="pmsg", bufs=2)
                    nc.tensor.matmul(pt[:], hT[:, sub * 128:(sub + 1) * 128],
                                     ident[:], start=True, stop=True)
                    nc.scalar.activation(stg[:, k], pt[:], AF.Relu)
                nc.sync.dma_start(
                    out.ap()[grp * 512:(grp + 1) * 512].rearrange(
                        "(a p) c -> p a c", p=128), stg[:])
    nc.compile()
    return nc


_CACHE = {}


def _build_cached(nops, opb2, nlayers=N_LAYERS):
    key = ("v3", nlayers, nops, tuple(opb2.reshape(-1).tolist()))
    if key not in _CACHE:
        _CACHE[key] = _build(nops, opb2, nlayers)
    return _CACHE[key]


def _make_inputs(x, edge_index, weight, w_ih, w_hh, b_ih, b_hh, nlayers):
    gsrc, dstrel, opb2 = _preprocess(edge_index)
    nops = int(opb2.sum())

    xpad = np.zeros((NCORES, PAD_SH, C), np.float32)
    xpad[:, :SHARD] = x.reshape(NCORES, SHARD, C)

    wstack = weight[:nlayers].transpose(1, 0, 2).reshape(C, nlayers * C).copy()
    wihT = w_ih.T.copy()                                   # [64, 192]
    whhT = w_hh.T.copy()
    brz = np.stack([(b_ih + b_hh)[0:C], (b_ih + b_hh)[C:2 * C]], 1)  # [64,2]
    bihn = b_ih[2 * C:3 * C].reshape(C, 1).copy()
    bhhn = b_hh[2 * C:3 * C].reshape(C, 1).copy()
    iota = np.tile(np.arange(128, dtype=np.float32), (128, 1))
    ident = np.eye(C, dtype=np.float32)

    in_maps = []
    for c in range(NCORES):
        in_maps.append({
            "xT": xpad[c].T.copy(), "wstack": wstack, "wihT": wihT,
            "whhT": whhT, "brz": brz, "bihn": bihn, "bhhn": bhhn,
            "iota": iota, "ident": ident,
            "gsrc": gsrc[c], "drel": dstrel[c],
        })
    return in_maps, nops, opb2


def _run(x, edge_index, weight, w_ih, w_hh, b_ih, b_hh,
         nlayers=N_LAYERS, trace=False):
    x = np.asarray(x, np.float32)
    weight = np.asarray(weight, np.float32)
    w_ih = np.asarray(w_ih, np.float32)
    w_hh = np.asarray(w_hh, np.float32)
    b_ih = np.asarray(b_ih, np.float32)
    b_hh = np.asarray(b_hh, np.float32)

    in_maps, nops, opb2 = _make_inputs(
        x, edge_index, weight, w_ih, w_hh, b_ih, b_hh, nlayers)
    nc = _build_cached(nops, opb2, nlayers)
    res = run_bass_kernel_spmd(nc, in_maps, core_ids=list(range(NCORES)),
                               trace=trace)
    h = np.stack([res.results[c]["hout"][:SHARD] for c in range(NCORES)])
    return h.reshape(N_NODES, C), res


def kernel(x, edge_index, weight, w_ih, w_hh, b_ih, b_hh):
    out, _ = _run(x, edge_index, weight, w_ih, w_hh, b_ih, b_hh)
    return out
